# revision 18
# baseline (speedup 1.0000x reference)
"""GCNConv (dense adjacency, 8192 nodes, 512 feat) on 8 Trainium2 NeuronCores.

Math (matches reference):
    A = adj + I
    deg = A.sum(axis=1); dinv = rsqrt(deg)        (deg >= 1 always)
    h = concat(x[:4096] @ Wr, x[4096:] @ Wd)
    out = leaky_relu(dinv[:,None] * (A @ (dinv[:,None] * h)) + bias, 0.01)

Sharding: rows of A / output row-sharded over 8 cores (1024 rows each).

The adjacency ships ONCE per core as fp8e4 DoubleRow packs [32, 128, 2048]
(A values are only 0/1/2 — exact in fp8):
  - the degree pass reads them in DoubleRow mode (2 j-rows/lane/cycle),
  - the main matmul reads [128, 512] strip-half slices of the same resident
    SBUF tile as its moving operand (bf16 stationary x fp8 moving runs at
    full bf16 rate on the PE and is exact for 0/1/2).
This halves adjacency HBM traffic vs shipping separate bf16 strips.

Each core scales its OWN h rows by its OWN dinv before the gather
(g = dinv * h), so no degree exchange is needed at all — consumers get
ready-to-use g. The gather is split into 4 chunked AllGathers; a host-side
j-permutation makes AG k deliver j-strips 16k..16k+15 in hg-slot order, so
the main matmul starts after the first chunk lands instead of after the
whole 8 MB of h.

Main matmul is computed transposed (out.T = g.T @ A.T): the stationary
operand is a [128,128] feature-chunk of g, the moving operand a [128,512]
fp8 strip-half — the epilogue's bias becomes per-partition and fuses into
the LeakyReLU activation, and the per-row dinv scaling is a free-axis
multiply against a broadcast tile built with a K=1 matmul.
"""

import numpy as np
import ml_dtypes

import concourse.bass as bass
import concourse.tile as tile
from concourse.masks import make_identity
from concourse import bacc, mybir
from concourse.bass_utils import run_bass_kernel_spmd

N = 8192
C = 512
NCORES = 8
ROWS = N // NCORES       # 1024 rows per core
P = 128
KT = N // P              # 64 contraction strips (permuted j order)
NPACK = KT // 2          # 32 fp8 DoubleRow packs (2 strips each)
MT = ROWS // P           # 8 own-row strips
FT = C // P              # 4 contraction tiles for x @ W
CC = C // P              # 4 feature chunks (stationary side of main matmul)
# chunked AllGathers: CH[q] = own strips contributed per core to AG q.
# Small first chunks let the main matmul start as soon as possible.
CH = [1, 1, 2, 2, 2]
CHB = [sum(CH[:q]) for q in range(len(CH))]     # local-strip base per chunk
KTAIL0 = 8 * (MT - CH[-1])   # first strip of the last chunk (per-cc tail)

F32 = mybir.dt.float32
BF16 = mybir.dt.bfloat16
FP8 = mybir.dt.float8e4

Lrelu = mybir.ActivationFunctionType.Lrelu
Rsqrt = mybir.ActivationFunctionType.Rsqrt


def _emit(nc, tc, dram, io, r, sim_mode=False):
    """Emit one full GCN pass. `r` tags pools/tiles for program repetition.

    sim_mode replaces collectives with local DMA stand-ins so the program
    can run under the single-core TimelineSim cost model."""
    adjt8_d, xt_d, w_d, biasc_d, out_d = io

    bounce = [dram.tile([ch * P, C], BF16, name=f"bounce{k}_{r}")
              for k, ch in enumerate(CH)]
    gfull = [dram.tile([NCORES * ch * P, C], BF16,
                       addr_space="Local" if sim_mode else "Shared",
                       name=f"gfull{k}_{r}")
             for k, ch in enumerate(CH)]
    dinvb = dram.tile([1, ROWS], F32, name=f"dinvb{r}")

    with tc.tile_pool(name=f"const{r}", bufs=1) as const_pool, \
         tc.tile_pool(name=f"adj{r}", bufs=1) as adj_pool, \
         tc.tile_pool(name=f"hbig{r}", bufs=1) as hbig_pool:
        ones8_t = const_pool.tile([P, 2, 16], FP8)
        nc.gpsimd.memset(ones8_t[:], 1.0)
        bias_pp = const_pool.tile([P, CC], F32)
        nc.sync.dma_start(bias_pp[:],
                          biasc_d.ap().rearrange("(cc p) -> p cc", p=P))
        dinvr_bc = const_pool.tile([P, ROWS], F32)
        dinv_col = const_pool.tile([P, MT], F32)
        hg_t = hbig_pool.tile([P, KT, C], BF16)
        adjt8_t = adj_pool.tile([P, NPACK, 2048], FP8)

        with tc.tile_pool(name=f"xw{r}", bufs=1) as xw_pool, \
             tc.tile_pool(name=f"hps{r}", bufs=2, space="PSUM") as hps_pool, \
             tc.tile_pool(name=f"hsb{r}", bufs=1) as hsb_pool, \
             tc.tile_pool(name=f"misc{r}", bufs=1) as misc_pool, \
             tc.tile_pool(name=f"degps{r}", bufs=1, space="PSUM") as degps_pool, \
             tc.tile_pool(name=f"tps{r}", bufs=1, space="PSUM") as tps_pool:
            # ---------------- input loads (order = DMA priority) -------
            xt_t = xw_pool.tile([P, FT, ROWS], BF16)
            nc.sync.dma_start(
                xt_t[:], xt_d.ap().rearrange("(f p) i -> p f i", p=P))
            w_t = xw_pool.tile([P, FT, C], BF16)
            nc.sync.dma_start(
                w_t[:], w_d.ap().rearrange("(f p) c -> p f c", p=P))
            for b in range(8):      # adjacency: 8 x 1MB
                nc.sync.dma_start(
                    adjt8_t[:, 4 * b:4 * (b + 1), :],
                    adjt8_d.ap()[4 * b:4 * (b + 1)].rearrange(
                        "s p i -> p s i"))

            # ---------------- Phase 1: h_shard = x_shard @ W -----------
            h_sb = hsb_pool.tile([P, MT, C], BF16)
            for mt in range(MT):
                h_ps = hps_pool.tile([P, C], F32, tag="hps")
                for ft in range(FT):
                    nc.tensor.matmul(
                        h_ps[:],
                        lhsT=xt_t[:, ft, mt * P:(mt + 1) * P],
                        rhs=w_t[:, ft, :],
                        start=(ft == 0), stop=(ft == FT - 1))
                nc.scalar.copy(h_sb[:, mt, :], h_ps[:])

            # ---------------- Phase 2: deg = row sums of A shard -------
            deg_ps = [degps_pool.tile([1, C], F32, tag=f"degp{i}",
                                      name=f"degp{i}")
                      for i in range(2)]
            for p in range(NPACK):
                r3 = adjt8_t[:, p, :].rearrange("p (two i) -> p two i", two=2)
                for half in range(2):
                    nc.tensor.matmul(
                        deg_ps[half][:],
                        lhsT=ones8_t[:, :, 0:1],
                        rhs=r3[:, :, half * C:(half + 1) * C],
                        perf_mode=mybir.MatmulPerfMode.DoubleRow,
                        start=(p == 0), stop=(p == NPACK - 1))
            # dinv for own rows, straight from PSUM: [1, 1024] free form
            rrow = misc_pool.tile([1, ROWS], F32, tag="rrow")
            for half in range(2):
                nc.vector.reciprocal(
                    rrow[:, half * C:(half + 1) * C], deg_ps[half][:])
            drow = misc_pool.tile([1, ROWS], F32, tag="drow")
            nc.scalar.sqrt(drow[:], rrow[:])

            # broadcast over partitions (for the output-row scaling)
            nc.gpsimd.partition_broadcast(dinvr_bc[:], drow[:])

            # [128, 8] column form (for scaling h rows): DRAM bounce to
            # reload as [8, 128], then PE-transpose. On the Act DGE queue
            # so it doesn't wait behind the adjacency stream.
            nc.scalar.dma_start(dinvb[:], drow[:])
            dk_sb = misc_pool.tile([MT, P], F32, tag="dk")
            nc.scalar.dma_start(
                dk_sb[:], dinvb.rearrange("a (m p) -> (a m) p", p=P))
            ident_t = misc_pool.tile([MT, MT], F32, tag="ident")
            make_identity(nc, ident_t[:])
            tp_ps = tps_pool.tile([P, MT], F32)
            nc.tensor.transpose(tp_ps[:], dk_sb[:], ident_t[:])
            nc.vector.tensor_copy(dinv_col[:], tp_ps[:])

            # ------------ Phase 3: g = dinv*h, chunked AllGather -------
            for k, ch in enumerate(CH):
                m0 = CHB[k]
                for m in range(m0, m0 + ch):
                    sl = h_sb[:, m, :]
                    nc.vector.tensor_scalar_mul(sl, sl,
                                                dinv_col[:, m:m + 1])
                nc.scalar.dma_start(
                    bounce[k].rearrange("(s p) c -> p s c", p=P),
                    h_sb[:, m0:m0 + ch, :])
                if sim_mode:
                    # local stand-in covering the full AG output so the
                    # compile-time schedule has the same dep structure
                    for c in range(NCORES):
                        nc.scalar.dma_start(
                            gfull[k][c * ch * P:(c + 1) * ch * P, :],
                            bounce[k][:])
                else:
                    nc.gpsimd.collective_compute(
                        "AllGather", mybir.AluOpType.bypass,
                        replica_groups=[list(range(NCORES))],
                        ins=[bounce[k].opt()], outs=[gfull[k].opt()])
                # reload in 512KB (4-strip) units so the matmul can start
                # on the first slots of a chunk before the rest land
                for b in range(2 * ch):
                    s0 = 8 * m0 + 4 * b
                    nc.scalar.dma_start(
                        hg_t[:, s0:s0 + 4, :],
                        gfull[k][4 * b * P:4 * (b + 1) * P, :].rearrange(
                            "(s p) c -> p s c", p=P))

        # ---------------- Phase 4+5: out.T = g.T @ A.T + epilogue ------
        with tc.tile_pool(name=f"mmps{r}", bufs=1, space="PSUM") as mmps_pool, \
             tc.tile_pool(name=f"ep{r}", bufs=4) as ep_pool:
            mm_ps = [mmps_pool.tile([P, ROWS], F32, tag=f"mm{cc}",
                                    name=f"mm{cc}")
                     for cc in range(CC)]

            def mm(cc, kt, start, stop):
                p, t = kt // 2, kt % 2
                for half in range(2):
                    off = t * 1024 + half * C
                    nc.tensor.matmul(
                        mm_ps[cc][:, half * C:(half + 1) * C],
                        lhsT=hg_t[:, kt, cc * P:(cc + 1) * P],
                        rhs=adjt8_t[:, p, off:off + C],
                        start=start, stop=stop)

            for kt in range(KTAIL0):
                for cc in range(CC):
                    mm(cc, kt, start=(kt == 0), stop=False)
            # trailing chunk per-cc so each chunk's epilogue overlaps the
            # next chunk's matmuls on the PE
            for cc in range(CC):
                for kt in range(KTAIL0, KT):
                    mm(cc, kt, start=False, stop=(kt == KT - 1))
                for eh in range(2):
                    sl = slice(eh * C, (eh + 1) * C)
                    t1 = ep_pool.tile([P, C], F32, tag="t1")
                    nc.vector.tensor_mul(t1[:], mm_ps[cc][:, sl],
                                         dinvr_bc[:, sl])
                    t2 = ep_pool.tile([P, C], F32, tag="t2")
                    nc.scalar.activation(
                        t2[:], t1[:], Lrelu,
                        bias=bias_pp[:, cc:cc + 1], alpha=0.01)
                    nc.sync.dma_start(
                        out_d.ap()[cc * P:(cc + 1) * P, sl], t2[:])


def build_kernel(reps: int = 1, sim_mode: bool = False):
    """Build and compile the SPMD Bass program (identical on all 8 cores).

    reps > 1 repeats the whole pipeline inside one NEFF (timing only)."""
    nc = bacc.Bacc("TRN2", target_bir_lowering=False, debug=False,
                   num_devices=NCORES)

    adjt8_d = nc.dram_tensor("adjt8", [NPACK, P, 2048], FP8,
                             kind="ExternalInput")
    xt_d = nc.dram_tensor("xt", [C, ROWS], BF16, kind="ExternalInput")
    w_d = nc.dram_tensor("w", [C, C], BF16, kind="ExternalInput")
    biasc_d = nc.dram_tensor("biasc", [C], F32, kind="ExternalInput")
    out_d = nc.dram_tensor("out", [C, ROWS], F32, kind="ExternalOutput")
    io = (adjt8_d, xt_d, w_d, biasc_d, out_d)

    with tile.TileContext(nc) as tc:
        with tc.tile_pool(name="dram", bufs=1, space="DRAM") as dram:
            if reps == 0:
                # near-empty program with the same I/O signature: used to
                # measure the dispatch floor
                with tc.tile_pool(name="nullp", bufs=1) as np_pool:
                    z = np_pool.tile([P, CC], F32, name="z")
                    nc.sync.dma_start(
                        z[:], biasc_d.ap().rearrange("(cc p) -> p cc", p=P))
            for r in range(reps):
                _emit(nc, tc, dram, io, r, sim_mode=sim_mode)

    nc.compile()
    return nc


def prepare_inputs(x, adj, weightr, weightd, bias):
    """Host-side sharding/layout. Returns in_maps for the 8 cores."""
    x = np.asarray(x, dtype=np.float32)
    adj = np.asarray(adj, dtype=np.float32)
    weightr = np.asarray(weightr, dtype=np.float32)
    weightd = np.asarray(weightd, dtype=np.float32)
    bias = np.ascontiguousarray(np.asarray(bias, dtype=np.float32))

    wr16 = weightr.astype(ml_dtypes.bfloat16)
    wd16 = weightd.astype(ml_dtypes.bfloat16)
    idx = np.arange(ROWS)
    # A values are only 0/1/2: build uint8 once, then LUT-cast (fast + exact)
    lut8 = np.array([0x00, 0x38, 0x40], dtype=np.uint8)          # e4m3 bits

    in_maps = []
    for c in range(NCORES):
        rows = slice(c * ROWS, (c + 1) * ROWS)
        ai = adj[rows, :].T.astype(np.uint8)             # [N, ROWS] 0/1
        ai[c * ROWS + idx, idx] += 1                     # fold in self-loop
        # j-permutation: chunked-AG q delivers, in hg-slot order, local
        # strips [CHB[q], CHB[q]+CH[q]) of cores 0..7
        a3 = ai.reshape(NCORES, MT, P, ROWS)             # [c', m, q, i]
        ap = np.concatenate(
            [a3[:, CHB[q]:CHB[q] + ch].reshape(NCORES * ch, P, ROWS)
             for q, ch in enumerate(CH)], axis=0)        # [KT, P, ROWS]
        # DoubleRow packs: pack p = new strips (2p, 2p+1), two j per lane
        adjt8 = np.ascontiguousarray(
            lut8[ap].view(ml_dtypes.float8_e4m3)
            .reshape(NPACK, 2, P, ROWS).transpose(0, 2, 1, 3)
        ).reshape(NPACK, P, 2048)
        xt = np.ascontiguousarray(x[rows, :].T).astype(ml_dtypes.bfloat16)
        w = wr16 if c < NCORES // 2 else wd16
        in_maps.append({"adjt8": adjt8, "xt": xt, "w": w, "biasc": bias})
    return in_maps


_NC_CACHE = {}


def kernel(x, adj, weightr, weightd, bias):
    if "nc" not in _NC_CACHE:
        _NC_CACHE["nc"] = build_kernel(reps=1)
    nc = _NC_CACHE["nc"]
    in_maps = prepare_inputs(x, adj, weightr, weightd, bias)
    res = run_bass_kernel_spmd(nc, in_maps, list(range(NCORES)))
    out = np.concatenate(
        [np.ascontiguousarray(res.results[c]["out"].T) for c in range(NCORES)],
        axis=0)
    return out
